# revision 1
# baseline (speedup 1.0000x reference)
"""BitNetLinear (ternary-quantized linear w/ training-blend) on 8 TRN2 NeuronCores.

Reference computation (fp32):
    thr  = mean(|W|)                       (global scalar over the full W)
    q    = sign(W) * (|W| > thr)           (ternary quantization)
    eff  = (1-l)*W + l*q, l=0.5            = 0.5*(W + q)
    eff  = eff * alpha
    out  = x @ eff^T + bias                x:[4,2048,4096] W:[4096,4096]

Sharding: tensor-parallel over out_features. Core c owns W rows
[c*512,(c+1)*512). x is replicated (pre-transposed to K-major bf16 on host),
W shard is shipped K-major in fp32 (threshold compare must see fp32 values).
Each core:
  - reduces |W_shard| -> scalar, 8-core AllReduce -> global threshold
  - quantizes+blends its shard on-device (fp32 math), casts to bf16 effT
    cached in SBUF in [K,O] layout
  - streams x^T tiles and runs 2048 bf16 matmuls (PSUM fp32 accumulation)
  - adds bias, writes its [8192, 512] fp32 output shard
Host concatenates the 8 shards along the output-feature axis.
"""

import numpy as np
import ml_dtypes

import concourse.bass as bass
import concourse.mybir as mybir
import concourse.tile as tile
from concourse import bacc
from concourse.bass_isa import ReduceOp
from concourse.bass_utils import run_bass_kernel_spmd

N_CORES = 8
CORE_IDS = list(range(N_CORES))

B, S, D_IN, D_OUT = 4, 2048, 4096, 4096
M = B * S                     # 8192 rows of x
O_SH = D_OUT // N_CORES       # 512 output features per core

P = 128                       # SBUF partitions
KO = D_IN // P                # 32 k-subtiles of 128
QCH = 4                       # k-subtiles per quantize chunk
NCH = KO // QCH               # 8 chunks
MT = 512                      # m-tile (x rows per output tile)
MS = MT // P                  # 4 PSUM subtiles per m-tile
NMT = M // MT                 # 16 m-tiles

_NC = None


def _build():
    dt = mybir.dt
    alu = mybir.AluOpType
    nc = bacc.Bacc("TRN2", target_bir_lowering=False, debug=False,
                   num_devices=N_CORES)

    xT = nc.dram_tensor("xT", [D_IN, M], dt.bfloat16, kind="ExternalInput").ap()
    wT = nc.dram_tensor("wT", [D_IN, O_SH], dt.float32, kind="ExternalInput").ap()
    bias_s = nc.dram_tensor("bias_s", [O_SH], dt.float32, kind="ExternalInput").ap()
    alpha_in = nc.dram_tensor("alpha_in", [1], dt.float32, kind="ExternalInput").ap()
    out = nc.dram_tensor("out", [M, O_SH], dt.float32, kind="ExternalOutput").ap()

    wT_r = wT.rearrange("(ko p) o -> p ko o", p=P)              # [128, 32, 512]
    xT_r = xT.rearrange("(ko p) m -> p ko m", p=P)              # [128, 32, 8192]
    out_r = out.rearrange("(mt ms p) o -> mt p ms o", p=P, ms=MS)

    with tile.TileContext(nc) as tc:
        with (
            tc.tile_pool(name="persist", bufs=1) as persist,
            tc.tile_pool(name="wstage", bufs=2) as wstage,
            tc.tile_pool(name="kxmp", bufs=2) as kxmp,
            tc.tile_pool(name="outp", bufs=3) as outp,
            tc.tile_pool(name="psum", bufs=2, space="PSUM") as psum,
            tc.tile_pool(name="dram", bufs=1, space="DRAM") as dram,
        ):
            # ---- runtime scalars, broadcast per-partition ----
            alpha_sb = persist.tile([1, 1], dt.float32)
            nc.sync.dma_start(alpha_sb[:], alpha_in[None, :])
            alpha_bc = persist.tile([P, 1], dt.float32)
            nc.gpsimd.partition_broadcast(alpha_bc[:], alpha_sb[:])
            c_p = persist.tile([P, 1], dt.float32)  # 0.5 * alpha
            nc.vector.tensor_scalar_mul(c_p[:], alpha_bc[:], 0.5)

            bias_row = persist.tile([1, O_SH], dt.float32)
            nc.sync.dma_start(bias_row[:], bias_s[None, :])
            bias_bc = persist.tile([P, O_SH], dt.float32)
            nc.gpsimd.partition_broadcast(bias_bc[:], bias_row[:])

            # ---- pass 1: partial sum of |W_shard| ----
            pp = persist.tile([P, KO], dt.float32)
            for g in range(NCH):
                wch = wstage.tile([P, QCH, O_SH], dt.float32, tag="wst",
                                  name=f"wch_a{g}")
                nc.sync.dma_start(wch[:], wT_r[:, g * QCH:(g + 1) * QCH, :])
                nc.vector.tensor_reduce(
                    pp[:, g * QCH:(g + 1) * QCH], wch[:],
                    axis=mybir.AxisListType.X, op=alu.add,
                    apply_absolute_value=True,
                )
            part1 = persist.tile([P, 1], dt.float32)
            nc.vector.tensor_reduce(part1[:], pp[:], axis=mybir.AxisListType.X,
                                    op=alu.add)
            red = persist.tile([P, 1], dt.float32)
            nc.gpsimd.partition_all_reduce(red[:], part1[:], P, ReduceOp.add)

            # ---- AllReduce partial sums across the 8 cores ----
            scal = persist.tile([1, 8], dt.float32)
            nc.vector.memset(scal[:], 0.0)
            nc.vector.tensor_copy(scal[:, 0:1], red[0:1, :])
            cc_in = dram.tile([1, 8], dt.float32)
            cc_out = dram.tile([1, 8], dt.float32, addr_space="Shared")
            nc.sync.dma_start(cc_in[:], scal[:])
            nc.gpsimd.collective_compute(
                "AllReduce", alu.add,
                ins=[cc_in.opt()], outs=[cc_out.opt()],
                replica_groups=[CORE_IDS],
            )
            tot_sb = persist.tile([1, 8], dt.float32)
            nc.sync.dma_start(tot_sb[:], cc_out[:])
            tot_bc = persist.tile([P, 8], dt.float32)
            nc.gpsimd.partition_broadcast(tot_bc[:], tot_sb[:])
            thr_p = persist.tile([P, 1], dt.float32)
            nc.vector.tensor_scalar_mul(thr_p[:], tot_bc[:, 0:1],
                                        1.0 / (D_OUT * D_IN))
            negthr_p = persist.tile([P, 1], dt.float32)
            nc.vector.tensor_scalar_mul(negthr_p[:], thr_p[:], -1.0)

            # ---- pass 2: quantize + blend -> effT bf16 [K, O] in SBUF ----
            # eff = c*w + c*((w > thr) - (w < -thr)),  c = 0.5*alpha
            effT = persist.tile([P, KO, O_SH], dt.bfloat16)
            for g in range(NCH):
                sl = slice(g * QCH, (g + 1) * QCH)
                wch = wstage.tile([P, QCH, O_SH], dt.float32, tag="wst",
                                  name=f"wch_b{g}")
                nc.sync.dma_start(wch[:], wT_r[:, sl, :])
                gc = wstage.tile([P, QCH, O_SH], dt.float32, tag="gc",
                                 name=f"gc{g}")
                nc.vector.tensor_scalar(
                    out=gc[:], in0=wch[:], scalar1=thr_p[:], scalar2=c_p[:],
                    op0=alu.is_gt, op1=alu.mult)
                lc = wstage.tile([P, QCH, O_SH], dt.float32, tag="lc",
                                 name=f"lc{g}")
                nc.vector.tensor_scalar(
                    out=lc[:], in0=wch[:], scalar1=negthr_p[:], scalar2=c_p[:],
                    op0=alu.is_lt, op1=alu.mult)
                nc.vector.tensor_tensor(gc[:], gc[:], lc[:], alu.subtract)
                nc.vector.scalar_tensor_tensor(
                    out=effT[:, sl, :], in0=wch[:], scalar=c_p[:], in1=gc[:],
                    op0=alu.mult, op1=alu.add)

            # ---- main matmul stream: out[m, o] = sum_k x[m,k] * eff[o,k] ----
            for mt in range(NMT):
                kxm = kxmp.tile([P, KO, MT], dt.bfloat16, tag="kxm",
                                name=f"kxm{mt}")
                msl = slice(mt * MT, (mt + 1) * MT)
                for g in range(NCH):
                    nc.sync.dma_start(
                        kxm[:, g * QCH:(g + 1) * QCH, :],
                        xT_r[:, g * QCH:(g + 1) * QCH, msl])
                pts = [psum.tile([P, O_SH], dt.float32, tag=f"ps{j}",
                                 name=f"ps{j}_{mt}") for j in range(MS)]
                for ko in range(KO):
                    for j in range(MS):
                        nc.tensor.matmul(
                            pts[j][:],
                            kxm[:, ko, j * P:(j + 1) * P],
                            effT[:, ko, :],
                            start=(ko == 0), stop=(ko == KO - 1))
                ot = outp.tile([P, MS, O_SH], dt.float32, tag="ot",
                               name=f"ot{mt}")
                for j in range(MS):
                    nc.vector.tensor_tensor(ot[:, j, :], pts[j][:], bias_bc[:],
                                            alu.add)
                nc.sync.dma_start(out_r[mt], ot[:])

    nc.compile()
    return nc


def _get_nc():
    global _NC
    if _NC is None:
        _NC = _build()
    return _NC


def kernel(x: np.ndarray, weight_fp: np.ndarray, bias: np.ndarray,
           alpha: np.ndarray, _trace: bool = False, **_kw):
    x = np.asarray(x)
    weight_fp = np.asarray(weight_fp, dtype=np.float32)
    bias = np.asarray(bias, dtype=np.float32)
    alpha = np.asarray(alpha, dtype=np.float32)

    # host-side layout prep: x -> K-major bf16 (replicated), W shard -> K-major fp32
    x2 = np.ascontiguousarray(
        x.reshape(M, D_IN).astype(ml_dtypes.bfloat16).T)       # [D_IN, M]
    in_maps = []
    for c in range(N_CORES):
        wsh = np.ascontiguousarray(
            weight_fp[c * O_SH:(c + 1) * O_SH, :].T)            # [D_IN, O_SH]
        in_maps.append({
            "xT": x2,
            "wT": wsh,
            "bias_s": np.ascontiguousarray(bias[c * O_SH:(c + 1) * O_SH]),
            "alpha_in": alpha,
        })

    nc = _get_nc()
    res = run_bass_kernel_spmd(nc, in_maps, CORE_IDS, trace=_trace)
    shards = [res.results[c]["out"] for c in range(N_CORES)]
    full = np.concatenate(shards, axis=1).reshape(B, S, D_OUT)
    if _trace:
        kernel.last_exec_time_ns = res.exec_time_ns
        kernel.last_results = res
    return full


if __name__ == "__main__":
    rng = np.random.default_rng(0)
    x = rng.standard_normal((B, S, D_IN), dtype=np.float32)
    w = rng.standard_normal((D_OUT, D_IN), dtype=np.float32)
    b = np.zeros(D_OUT, np.float32)
    a = np.ones(1, np.float32)
    out = kernel(x, w, b, a)
    print("out", out.shape, out.dtype, out[0, 0, :4])
